# revision 10
# baseline (speedup 1.0000x reference)
"""K-center kernel: argmax_i min_j ||A_i - B_j|| on 8 NeuronCores.

Strategy (prune + rescue):
  Device pass over a SUBSET of B: the Ms=512 columns with smallest
  ||b||^2 (sorted ascending).  For each row i this yields
      ub_x[i] ~= min_{j in subset} (||b_j||^2 - 2 a_i . b_j)
  which upper-bounds the full min over B.  Host then scans rows in
  descending ub order, rescoring each EXACTLY (float64, full B) and
  stops as soon as the next row's ub + EPS1 cannot beat the best exact
  value seen — sound because m_i <= ub_i <= ub_dev_i + EPS1.

  Device (per core: 6250 rows, 49 row-tiles of 128; fp8 DoubleRow
  matmuls, 2 per row-tile, K=256 each, into [128,2048] 4-bank PSUM
  tiles holding 4 row-tiles):
    - Row-tiles are processed in blocks of 4: three "V" tiles reduced
      by the Vector engine (one batched grouped tensor_reduce(min) over
      16 nb-groups of 32 columns x 3 tiles, add nb midpoints, min),
      one "S" tile reduced by the Scalar engine via softmin:
        S = sum_j exp((SHIFT - (p_ij + nb_mid))/T)   (Exp + accumulate)
      host recovers m_S = SHIFT - T*ln(S) <= true subset min.
  EPS1 absorbs all device-vs-exact error (fp8 rounding, nb group
  midpoints, softmin slack); the final answer is exact because every
  returned (idx, val) comes from the float64 host rescore.
"""

import numpy as np
import ml_dtypes

N_CORES = 8
N_TOTAL = 50000
M_B = 5000
D_FEAT = 512
N_PER_CORE = N_TOTAL // N_CORES          # 6250
ROW_TILES = 49                            # ceil(6250/128)
N_PAD = ROW_TILES * 128                   # 6272
MS = 512                                  # subset size (smallest nb)
GRP_V = 32                                # V-path nb group size (16 groups)
N_BLOCKS = 12                             # blocks of 4 tiles (V,V,V,S); tile 48 V

SOFT_T = 2.25                             # softmin temperature (sq units)
SOFT_SHIFT = 250.0                        # exp arg shift (sq units)
EPS1 = 0.85                               # ub_dev underestimate allowance (D units)

# S-tiles: last two tiles of each 4-block (2V + 2S balance)
S_TILES = sorted(
    [4 * b + 2 for b in range(N_BLOCKS)] + [4 * b + 3 for b in range(N_BLOCKS)]
)                                                               # 24
V_TILES = [it for it in range(ROW_TILES) if it not in S_TILES]  # 25

_compiled = None


def build_program():
    import concourse.tile as tile
    import concourse.mybir as mybir
    from concourse import bacc

    nc = bacc.Bacc("TRN2", target_bir_lowering=False, debug=False)
    atb = nc.dram_tensor(
        "ATB", [ROW_TILES, 128, 512], mybir.dt.float8e4, kind="ExternalInput"
    ).ap()
    btb = nc.dram_tensor(
        "BTB", [128, 4 * MS], mybir.dt.float8e4, kind="ExternalInput"
    ).ap()
    nbg = nc.dram_tensor(
        "NBG", [128, 2 * (MS // GRP_V)], mybir.dt.float32, kind="ExternalInput"
    ).ap()
    nbb = nc.dram_tensor(
        "NBB", [128, 1], mybir.dt.float32, kind="ExternalInput"
    ).ap()
    mout = nc.dram_tensor(
        "OUT", [128, ROW_TILES], mybir.dt.float32, kind="ExternalOutput"
    ).ap()

    fp32 = mybir.dt.float32
    fp8 = mybir.dt.float8e4
    bf16 = mybir.dt.bfloat16
    DR = mybir.MatmulPerfMode.DoubleRow
    add = mybir.AluOpType.add
    amin = mybir.AluOpType.min
    X = mybir.AxisListType.X
    Exp = mybir.ActivationFunctionType.Exp

    ng = MS // GRP_V                      # 16 groups per tile

    with tile.TileContext(nc) as tc:
        with (
            tc.tile_pool(name="const", bufs=1) as cpool,
            tc.tile_pool(name="psum", bufs=2, space="PSUM") as pspool,
            tc.tile_pool(name="fin", bufs=3) as fpool,
            tc.tile_pool(name="scr", bufs=2) as scrpool,
            tc.tile_pool(name="out", bufs=1) as mpool,
        ):
            a_all = cpool.tile([128, ROW_TILES * 512], fp8)
            bt_sb = cpool.tile([128, 4 * MS], fp8)       # [p, kt(2), half(2), j]
            nbg_sb = cpool.tile([128, 2 * ng], fp32)
            nbb_sb = cpool.tile([128, 1], fp32)
            warm_sb = cpool.tile([128, 1], fp32)
            m_sb = mpool.tile([128, ROW_TILES], fp32)

            # Startup DMAs spread over four queues so the first block's data
            # (A tile 0, B kt0) lands ~1.3us after the preamble instead of
            # serializing on one queue. Remaining A arrives in 4-tile groups
            # alternating sync/gpsimd, matching block consumption order.
            nc.sync.dma_start(out=a_all[:, 0:512], in_=atb[0])
            nc.gpsimd.dma_start(out=bt_sb[:, 0 : 2 * MS], in_=btb[:, 0 : 2 * MS])
            nc.scalar.dma_start(out=nbg_sb[:], in_=nbg[:])
            nc.scalar.dma_start(out=nbb_sb[:], in_=nbb[:])
            nc.scalar.dma_start(
                out=bt_sb[:, 2 * MS : 4 * MS], in_=btb[:, 2 * MS : 4 * MS]
            )
            # Early dummy Exp so the activation table load (~2.7us) overlaps
            # the initial DMA instead of stalling the first real softmin.
            nc.scalar.activation(out=warm_sb[:], in_=nbb_sb[:], func=Exp)
            for gi, lo in enumerate(range(1, ROW_TILES, 4)):
                hi = min(lo + 4, ROW_TILES)
                eng = nc.sync if gi % 2 == 0 else nc.gpsimd
                eng.dma_start(
                    out=a_all[:, lo * 512 : hi * 512].rearrange(
                        "p (g f) -> p g f", g=hi - lo
                    ),
                    in_=atb[lo:hi].rearrange("g p f -> p g f"),
                )

            bt_v = bt_sb[:].rearrange("p (kt two j) -> p kt two j", kt=2, two=2)

            def mm(ps_slice, it, kt):
                lhsT3 = a_all[
                    :, it * 512 + kt * 256 : it * 512 + (kt + 1) * 256
                ].rearrange("p (two f) -> p two f", two=2)
                nc.tensor.matmul(
                    ps_slice,
                    lhsT=lhsT3,
                    rhs=bt_v[:, kt, :, :],
                    start=(kt == 0),
                    stop=(kt == 1),
                    perf_mode=DR,
                )

            for b in range(N_BLOCKS):
                ps = pspool.tile([128, 2048], fp32)
                for t in range(4):
                    for kt in range(2):
                        mm(ps[:, t * 512 : (t + 1) * 512], 4 * b + t, kt)
                # V path: tiles 4b, 4b+1 in one batched grouped reduce
                gm = fpool.tile([128, 2 * ng], fp32)
                nc.vector.tensor_reduce(
                    out=gm[:],
                    in_=ps[:, 0:1024].rearrange("p (t a c) -> p t a c", t=2, c=GRP_V),
                    axis=X,
                    op=amin,
                )
                sm = fpool.tile([128, 2 * ng], fp32)
                nc.vector.tensor_tensor(out=sm[:], in0=gm[:], in1=nbg_sb[:], op=add)
                nc.vector.tensor_reduce(
                    out=m_sb[:, 4 * b : 4 * b + 2],
                    in_=sm[:].rearrange("p (t a) -> p t a", a=ng),
                    axis=X,
                    op=amin,
                )
                # S path: tiles 4b+2, 4b+3 softmin on the Scalar engine
                for t in (2, 3):
                    scr = scrpool.tile([128, 512], bf16)
                    nc.scalar.activation(
                        out=scr[:],
                        in_=ps[:, t * 512 : (t + 1) * 512],
                        func=Exp,
                        bias=nbb_sb[:, 0:1],
                        scale=-1.0 / SOFT_T,
                        accum_out=m_sb[:, 4 * b + t : 4 * b + t + 1],
                    )
            # lone tile 48 (V path)
            ps = pspool.tile([128, 2048], fp32)
            for kt in range(2):
                mm(ps[:, 0:512], 48, kt)
            gm = fpool.tile([128, ng], fp32)
            nc.vector.tensor_reduce(
                out=gm[:],
                in_=ps[:, 0:512].rearrange("p (a c) -> p a c", c=GRP_V),
                axis=X,
                op=amin,
            )
            sm = fpool.tile([128, ng], fp32)
            nc.vector.tensor_tensor(out=sm[:], in0=gm[:], in1=nbg_sb[:, 0:ng], op=add)
            nc.vector.tensor_reduce(
                out=m_sb[:, 48:49], in_=sm[:], axis=X, op=amin
            )
            nc.sync.dma_start(out=mout[:], in_=m_sb[:])
    nc.compile()
    return nc


def prep_inputs(A, B):
    """Returns atb [8,49,128,512] fp8, btb [128,4*MS] fp8, nbg, nbb."""
    e4 = ml_dtypes.float8_e4m3
    B32 = B.astype(np.float32)
    nb = (B32.astype(np.float64) ** 2).sum(axis=1)
    order = np.argsort(nb, kind="stable")[:MS]
    Bs = B32[order]                       # [MS, 512] ascending nb
    nbs = nb[order]

    # ATB: per-core row-tile blocks [core, 49, 128p(feat%128), 512] of -2A
    Apad = np.zeros((N_CORES, N_PAD, D_FEAT), np.float32)
    Apad[:, :N_PER_CORE, :] = (-2.0 * A.astype(np.float32)).reshape(
        N_CORES, N_PER_CORE, D_FEAT
    )
    # feature index = kt*256 + half*128 + p
    atb = (
        np.ascontiguousarray(
            Apad.reshape(N_CORES, ROW_TILES, 128, 2, 2, 128).transpose(
                0, 1, 5, 3, 4, 2
            )
        )
        .reshape(N_CORES, ROW_TILES, 128, 512)
        .astype(e4)
    )

    # BTB: [128p, kt(2), half(2), j] = Bs[j, kt*256+half*128+p]
    btb = (
        np.ascontiguousarray(Bs.reshape(MS, 2, 2, 128).transpose(3, 1, 2, 0))
        .reshape(128, 4 * MS)
        .astype(e4)
    )

    # V-path per-group nb midpoints (16 groups of 32), replicated x2 tiles
    g = nbs.reshape(MS // GRP_V, GRP_V)
    mid_v = ((g.min(axis=1) + g.max(axis=1)) * 0.5).astype(np.float32)
    nbg = np.ascontiguousarray(
        np.broadcast_to(np.tile(mid_v, 2)[None, :], (128, 2 * (MS // GRP_V)))
    ).astype(np.float32)

    # S-path single-span bias: (SHIFT - nb_mid_all) / T
    mid_all = (nbs.min() + nbs.max()) * 0.5
    bias = np.float32((SOFT_SHIFT - mid_all) / SOFT_T)
    nbb = np.full((128, 1), bias, np.float32)
    return atb, btb, nbg, nbb


def _assemble_ub(res):
    """Per-core OUT -> ub_x for all 50000 rows (device estimate)."""
    ub = np.empty(N_TOTAL, np.float64)
    s_cols = np.zeros(ROW_TILES, bool)
    s_cols[S_TILES] = True
    with np.errstate(divide="ignore"):
        for c in range(N_CORES):
            o = res.results[c]["OUT"].astype(np.float64)     # [128, 49]
            rows = np.where(
                s_cols[None, :], SOFT_SHIFT - SOFT_T * np.log(o), o
            )                                                 # [128, 49]
            flat = rows.T.reshape(-1)[:N_PER_CORE]
            ub[c * N_PER_CORE : (c + 1) * N_PER_CORE] = flat
    return ub


def _scan_rescore(A, B, ub_d):
    """Exact scan in descending device-ub order with sound stop rule."""
    A64 = A.astype(np.float64)
    B64 = B.astype(np.float64)
    nb = (B64**2).sum(axis=1)[None, :]
    order = np.argsort(-ub_d, kind="stable")
    best_val = -np.inf
    best_idx = -1
    pos = 0
    BATCH = 128
    n_scanned = 0
    while pos < N_TOTAL:
        if pos >= 8 and ub_d[order[pos]] + EPS1 < best_val:
            break
        idx = order[pos : pos + BATCH]
        Ab = A64[idx]
        na = (Ab**2).sum(axis=1)[:, None]
        sq = na - 2.0 * (Ab @ B64.T) + nb
        d = np.sqrt(np.maximum(sq, 0.0)).min(axis=1)
        w = int(np.argmax(d))
        if d[w] > best_val:
            best_val = float(d[w])
            best_idx = int(idx[w])
        n_scanned += len(idx)
        pos += BATCH
    return best_idx, best_val, n_scanned


def kernel(A, B, _trace=False):
    from concourse.bass_utils import run_bass_kernel_spmd

    global _compiled
    if _compiled is None:
        _compiled = build_program()
    nc = _compiled

    A = np.asarray(A, np.float32)
    B = np.asarray(B, np.float32)
    atb, btb, nbg, nbb = prep_inputs(A, B)
    in_maps = [
        {"ATB": atb[c], "BTB": btb, "NBG": nbg, "NBB": nbb}
        for c in range(N_CORES)
    ]
    res = run_bass_kernel_spmd(nc, in_maps, list(range(N_CORES)), trace=_trace)

    ub_x = _assemble_ub(res)
    na = (A.astype(np.float64) ** 2).sum(axis=1)
    with np.errstate(invalid="ignore"):
        ub_d = np.sqrt(np.maximum(na + ub_x, 0.0))
    ub_d = np.where(np.isnan(ub_d), np.inf, ub_d)

    idx, val, n_scanned = _scan_rescore(A, B, ub_d)
    out = (np.array(idx, dtype=np.int32), np.array(val, dtype=np.float32))
    if _trace:
        return out, res, ub_d, n_scanned
    return out


# revision 18
# speedup vs baseline: 1.2303x; 1.2303x over previous
"""K-center kernel: argmax_i min_j ||A_i - B_j|| on 8 NeuronCores.

Strategy (prune + rescue):
  Device pass over a SUBSET of B: the Ms=512 columns with smallest
  ||b||^2 (sorted ascending).  For each row i this yields
      ub_x[i] ~= min_{j in subset} (||b_j||^2 - 2 a_i . b_j)
  which upper-bounds the full min over B.  Host then scans rows in
  descending ub order, rescoring each EXACTLY (float64, full B) and
  stops as soon as the next row's ub + EPS1 cannot beat the best exact
  value seen — sound because m_i <= ub_i <= ub_dev_i + EPS1.

  Device (per core: 6250 rows, 49 row-tiles of 128; fp8 DoubleRow
  matmuls, 2 per row-tile, K=256 each, into [128,2048] 4-bank PSUM
  tiles holding 4 row-tiles):
    - Row-tiles are processed in blocks of 4: three "V" tiles reduced
      by the Vector engine (one batched grouped tensor_reduce(min) over
      16 nb-groups of 32 columns x 3 tiles, add nb midpoints, min),
      one "S" tile reduced by the Scalar engine via softmin:
        S = sum_j exp((SHIFT - (p_ij + nb_mid))/T)   (Exp + accumulate)
      host recovers m_S = SHIFT - T*ln(S) <= true subset min.
  EPS1 absorbs all device-vs-exact error (fp8 rounding, nb group
  midpoints, softmin slack); the final answer is exact because every
  returned (idx, val) comes from the float64 host rescore.
"""

import numpy as np
import ml_dtypes

N_CORES = 8
N_TOTAL = 50000
M_B = 5000
D_FEAT = 512
N_PER_CORE = N_TOTAL // N_CORES          # 6250
ROW_TILES = 49                            # ceil(6250/128)
N_PAD = ROW_TILES * 128                   # 6272
MS = 512                                  # subset size (smallest nb)
GRP_V = 32                                # V-path nb group size (16 groups)
N_BLOCKS = 12                             # blocks of 4 tiles (V,V,V,S); tile 48 V

SOFT_T = 2.25                             # softmin temperature (sq units)
SOFT_SHIFT = 250.0                        # exp arg shift (sq units)
EPS1 = 0.85                               # ub_dev underestimate allowance (D units)

# S-tiles: last two tiles of each 4-block (2V + 2S balance)
S_TILES = sorted(
    [4 * b + 2 for b in range(N_BLOCKS)] + [4 * b + 3 for b in range(N_BLOCKS)]
)                                                               # 24
V_TILES = [it for it in range(ROW_TILES) if it not in S_TILES]  # 25

_compiled = None


def build_program():
    import concourse.tile as tile
    import concourse.mybir as mybir
    from concourse import bacc

    nc = bacc.Bacc("TRN2", target_bir_lowering=False, debug=False)
    atb = nc.dram_tensor(
        "ATB", [128, ROW_TILES * 512], mybir.dt.float8e4, kind="ExternalInput"
    ).ap()
    btb = nc.dram_tensor(
        "BTB", [128, 4 * MS], mybir.dt.float8e4, kind="ExternalInput"
    ).ap()
    nbg = nc.dram_tensor(
        "NBG", [128, 2 * (MS // GRP_V)], mybir.dt.float32, kind="ExternalInput"
    ).ap()
    nbb = nc.dram_tensor(
        "NBB", [128, 1], mybir.dt.float32, kind="ExternalInput"
    ).ap()
    mout = nc.dram_tensor(
        "OUT", [128, ROW_TILES], mybir.dt.float32, kind="ExternalOutput"
    ).ap()

    fp32 = mybir.dt.float32
    fp8 = mybir.dt.float8e4
    bf16 = mybir.dt.bfloat16
    DR = mybir.MatmulPerfMode.DoubleRow
    add = mybir.AluOpType.add
    amin = mybir.AluOpType.min
    X = mybir.AxisListType.X
    Exp = mybir.ActivationFunctionType.Exp

    ng = MS // GRP_V                      # 16 groups per tile

    with tile.TileContext(nc) as tc:
        with (
            tc.tile_pool(name="const", bufs=1) as cpool,
            tc.tile_pool(name="psv", bufs=2, space="PSUM") as psvpool,
            tc.tile_pool(name="pss", bufs=3, space="PSUM") as psspool,
            tc.tile_pool(name="fin", bufs=3) as fpool,
            tc.tile_pool(name="scr", bufs=2) as scrpool,
            tc.tile_pool(name="out", bufs=1) as mpool,
        ):
            a_all = cpool.tile([128, ROW_TILES * 512], fp8)
            bt_sb = cpool.tile([128, 4 * MS], fp8)       # [p, kt(2), half(2), j]
            nbg_sb = cpool.tile([128, 2 * ng], fp32)
            nbb_sb = cpool.tile([128, 1], fp32)
            warm_sb = cpool.tile([128, 1], fp32)
            m_sb = mpool.tile([128, ROW_TILES], fp32)

            # Startup DMAs: B kt0 / B kt1 / A tile0 issue first on three
            # separate queues (the first matmul gates on all three), then
            # the rest of A arrives as three ~1MB partition-major descriptors
            # (large contiguous per-partition runs DMA much faster than
            # per-tile 512B-run transfers).
            nc.sync.dma_start(out=bt_sb[:, 0 : 2 * MS], in_=btb[:, 0 : 2 * MS])
            nc.gpsimd.dma_start(
                out=bt_sb[:, 2 * MS : 4 * MS], in_=btb[:, 2 * MS : 4 * MS]
            )
            nc.scalar.dma_start(out=a_all[:, 0:512], in_=atb[:, 0:512])
            nc.scalar.dma_start(out=nbg_sb[:], in_=nbg[:])
            nc.scalar.dma_start(out=nbb_sb[:], in_=nbb[:])
            # Early dummy Exp so the activation table load (~2.7us) overlaps
            # the initial DMA instead of stalling the first real softmin.
            nc.scalar.activation(out=warm_sb[:], in_=nbb_sb[:], func=Exp)
            for eng, lo, hi in (
                (nc.sync, 1, 17),
                (nc.gpsimd, 17, 33),
                (nc.scalar, 33, ROW_TILES),
            ):
                eng.dma_start(
                    out=a_all[:, lo * 512 : hi * 512],
                    in_=atb[:, lo * 512 : hi * 512],
                )

            bt_v = bt_sb[:].rearrange("p (kt two j) -> p kt two j", kt=2, two=2)

            def mm(ps_slice, it, kt):
                lhsT3 = a_all[
                    :, it * 512 + kt * 256 : it * 512 + (kt + 1) * 256
                ].rearrange("p (two f) -> p two f", two=2)
                nc.tensor.matmul(
                    ps_slice,
                    lhsT=lhsT3,
                    rhs=bt_v[:, kt, :, :],
                    start=(kt == 0),
                    stop=(kt == 1),
                    perf_mode=DR,
                )

            for b in range(N_BLOCKS):
                psv = psvpool.tile([128, 1024], fp32, tag="psv")
                for t in range(2):
                    for kt in range(2):
                        mm(psv[:, t * 512 : (t + 1) * 512], 4 * b + t, kt)
                pss_tiles = []
                for t in (2, 3):
                    pss = psspool.tile([128, 512], fp32, tag="pss")
                    pss_tiles.append(pss)
                    for kt in range(2):
                        mm(pss[:], 4 * b + t, kt)
                # V path: tiles 4b, 4b+1 in one batched grouped reduce
                gm = fpool.tile([128, 2 * ng], fp32)
                nc.vector.tensor_reduce(
                    out=gm[:],
                    in_=psv[:].rearrange("p (t a c) -> p t a c", t=2, c=GRP_V),
                    axis=X,
                    op=amin,
                )
                sm = fpool.tile([128, 2 * ng], fp32)
                nc.vector.tensor_tensor(out=sm[:], in0=gm[:], in1=nbg_sb[:], op=add)
                nc.vector.tensor_reduce(
                    out=m_sb[:, 4 * b : 4 * b + 2],
                    in_=sm[:].rearrange("p (t a) -> p t a", a=ng),
                    axis=X,
                    op=amin,
                )
                # S path: tiles 4b+2, 4b+3 softmin on the Scalar engine
                for k, t in enumerate((2, 3)):
                    scr = scrpool.tile([128, 512], bf16)
                    nc.scalar.activation(
                        out=scr[:],
                        in_=pss_tiles[k][:],
                        func=Exp,
                        bias=nbb_sb[:, 0:1],
                        scale=-1.0 / SOFT_T,
                        accum_out=m_sb[:, 4 * b + t : 4 * b + t + 1],
                    )
            # lone tile 48 (V path)
            ps48 = psvpool.tile([128, 1024], fp32, tag="psv")
            for kt in range(2):
                mm(ps48[:, 0:512], 48, kt)
            gm = fpool.tile([128, ng], fp32)
            nc.vector.tensor_reduce(
                out=gm[:],
                in_=ps48[:, 0:512].rearrange("p (a c) -> p a c", c=GRP_V),
                axis=X,
                op=amin,
            )
            sm = fpool.tile([128, ng], fp32)
            nc.vector.tensor_tensor(out=sm[:], in0=gm[:], in1=nbg_sb[:, 0:ng], op=add)
            nc.vector.tensor_reduce(
                out=m_sb[:, 48:49], in_=sm[:], axis=X, op=amin
            )
            nc.sync.dma_start(out=mout[:], in_=m_sb[:])
    nc.compile()
    return nc


def prep_inputs(A, B):
    """Returns atb [8,49,128,512] fp8, btb [128,4*MS] fp8, nbg, nbb."""
    e4 = ml_dtypes.float8_e4m3
    B32 = B.astype(np.float32)
    nb = (B32.astype(np.float64) ** 2).sum(axis=1)
    order = np.argsort(nb, kind="stable")[:MS]
    Bs = B32[order]                       # [MS, 512] ascending nb
    nbs = nb[order]

    # ATB: partition-major [core, 128p(feat%128), (tile, kt, half, row)] of
    # -2A, so each partition's data for a span of tiles is one contiguous
    # run (fast DMA). feature index = kt*256 + half*128 + p.
    Apad = np.zeros((N_CORES, N_PAD, D_FEAT), np.float32)
    Apad[:, :N_PER_CORE, :] = (-2.0 * A.astype(np.float32)).reshape(
        N_CORES, N_PER_CORE, D_FEAT
    )
    atb = (
        np.ascontiguousarray(
            Apad.reshape(N_CORES, ROW_TILES, 128, 2, 2, 128).transpose(
                0, 5, 1, 3, 4, 2
            )
        )
        .reshape(N_CORES, 128, ROW_TILES * 512)
        .astype(e4)
    )

    # BTB: [128p, kt(2), half(2), j] = Bs[j, kt*256+half*128+p]
    btb = (
        np.ascontiguousarray(Bs.reshape(MS, 2, 2, 128).transpose(3, 1, 2, 0))
        .reshape(128, 4 * MS)
        .astype(e4)
    )

    # V-path per-group nb midpoints (16 groups of 32), replicated x2 tiles
    g = nbs.reshape(MS // GRP_V, GRP_V)
    mid_v = ((g.min(axis=1) + g.max(axis=1)) * 0.5).astype(np.float32)
    nbg = np.ascontiguousarray(
        np.broadcast_to(np.tile(mid_v, 2)[None, :], (128, 2 * (MS // GRP_V)))
    ).astype(np.float32)

    # S-path single-span bias: (SHIFT - nb_mid_all) / T
    mid_all = (nbs.min() + nbs.max()) * 0.5
    bias = np.float32((SOFT_SHIFT - mid_all) / SOFT_T)
    nbb = np.full((128, 1), bias, np.float32)
    return atb, btb, nbg, nbb


def _assemble_ub(res):
    """Per-core OUT -> ub_x for all 50000 rows (device estimate)."""
    ub = np.empty(N_TOTAL, np.float64)
    s_cols = np.zeros(ROW_TILES, bool)
    s_cols[S_TILES] = True
    with np.errstate(divide="ignore"):
        for c in range(N_CORES):
            o = res.results[c]["OUT"].astype(np.float64)     # [128, 49]
            rows = np.where(
                s_cols[None, :], SOFT_SHIFT - SOFT_T * np.log(o), o
            )                                                 # [128, 49]
            flat = rows.T.reshape(-1)[:N_PER_CORE]
            ub[c * N_PER_CORE : (c + 1) * N_PER_CORE] = flat
    return ub


def _scan_rescore(A, B, ub_d):
    """Exact scan in descending device-ub order with sound stop rule."""
    A64 = A.astype(np.float64)
    B64 = B.astype(np.float64)
    nb = (B64**2).sum(axis=1)[None, :]
    order = np.argsort(-ub_d, kind="stable")
    best_val = -np.inf
    best_idx = -1
    pos = 0
    BATCH = 128
    n_scanned = 0
    while pos < N_TOTAL:
        if pos >= 8 and ub_d[order[pos]] + EPS1 < best_val:
            break
        idx = order[pos : pos + BATCH]
        Ab = A64[idx]
        na = (Ab**2).sum(axis=1)[:, None]
        sq = na - 2.0 * (Ab @ B64.T) + nb
        d = np.sqrt(np.maximum(sq, 0.0)).min(axis=1)
        w = int(np.argmax(d))
        if d[w] > best_val:
            best_val = float(d[w])
            best_idx = int(idx[w])
        n_scanned += len(idx)
        pos += BATCH
    return best_idx, best_val, n_scanned


def kernel(A, B, _trace=False):
    from concourse.bass_utils import run_bass_kernel_spmd

    global _compiled
    if _compiled is None:
        _compiled = build_program()
    nc = _compiled

    A = np.asarray(A, np.float32)
    B = np.asarray(B, np.float32)
    atb, btb, nbg, nbb = prep_inputs(A, B)
    in_maps = [
        {"ATB": atb[c], "BTB": btb, "NBG": nbg, "NBB": nbb}
        for c in range(N_CORES)
    ]
    res = run_bass_kernel_spmd(nc, in_maps, list(range(N_CORES)), trace=_trace)

    ub_x = _assemble_ub(res)
    na = (A.astype(np.float64) ** 2).sum(axis=1)
    with np.errstate(invalid="ignore"):
        ub_d = np.sqrt(np.maximum(na + ub_x, 0.0))
    ub_d = np.where(np.isnan(ub_d), np.inf, ub_d)

    idx, val, n_scanned = _scan_rescore(A, B, ub_d)
    out = (np.array(idx, dtype=np.int32), np.array(val, dtype=np.float32))
    if _trace:
        return out, res, ub_d, n_scanned
    return out
